# revision 88
# baseline (speedup 1.0000x reference)
"""Trainium2 Bass kernel for nn_MultiHeadCausalTensionLayer.

Reference computation (B=1, T=2048, D=1024, H=16, HD=64, WN=64):
  q,k,v = x@wq, x@wk, x@wv  (per-head RoPE on q,k)
  scores[t,h,w] = q[t,h]·k[t-64+w,h] / 8          (w in [0,64), causal window)
  tau = sigmoid(scores) * causal_mask
  msg = (tau @ window_v) / clip(sum_w tau, 1e-6)
  out = rms_norm(x + msg.flat @ wo) * norm_scale

Sharding: sequence-parallel over T across 8 cores (256 rows each) with a
64-row halo; the halo is materialized host-side (zero-padded for core 0),
so each core's program is identical, fully local, and needs no collectives.

v2 design vs the bf16 baseline (110us -> 78us measured by loop-delta):
- All four projections (q/k/v/wo) run as fp8-e4m3 DoubleRow matmuls
  (2 contraction chunks per instruction, 0.5 cycles/row = 2x bf16 PE
  throughput; fp8 weights also halve the weight DMA). Attention math
  (scores, tau, msg) stays bf16; end-to-end maxrel ~1.5e-2 < 2e-2.
- Host-side pre-transposed weight layout [p, k, d] so weight DMAs are
  fully contiguous (the strided rearrange cost ~14us in 1KB-descriptor
  overhead); x ships as bf16 for the transpose path + f32 for the
  residual.
- Score/msg/mass matmuls are trimmed to the causal band: query block
  [0:128) only touches key block 0, [64:256) block 1, [192:256) block 2.
  Score tile is [P, hh, 512-padded] so the two heads of a pair land in
  different PSUM banks (they run concurrently on disjoint PE row groups
  0:64 / 64:128 and must not share a bank). PSUM start=True marks the
  whole 2KB-per-partition granule pending-zero, so each accumulation
  group sets start exactly once (lazily zeroing later partial ranges).
- Lookahead software pipeline: iteration c runs rope(c-1) from the
  previous iteration's projection copy, projection(c), msg+mass(c-2),
  and scores/sigmoid/mask(c-1). No stage waits on a same-iteration
  cross-engine round trip, keeping the PE busy and its p-state clock up
  (HW drops the PE clock 2x after idle gaps).
- One fused sigmoid + one fused mask op per pair over the banded tau
  tile (the unwritten psum corner is finite garbage zeroed by the mask).
- ACT table preloads: a dummy sigmoid before any real ACT op loads the
  sigmoid set during the initial DMA wait; a dummy sqrt that READS the
  last tau (so the scheduler cannot hoist it) loads the sqrt set off the
  output-phase critical path.
"""

import numpy as np

import concourse.bass as bass
import concourse.mybir as mybir
import concourse.tile as tile
from concourse import bacc, bass_utils

# Problem constants (hardcoded per harness contract).
B, T, D = 1, 2048, 1024
H, HD, WN = 16, 64, 64
ROPE_BASE = 10000.0
EPS = 1e-6
NCORES = 8
TLOC = T // NCORES          # 256 rows per core
TEXT = TLOC + WN            # 320 rows incl. halo
P = 128
KCH = D // P                # 8 contraction chunks
MCH = D // P                # 8 output chunks
NKB = TEXT // P + (1 if TEXT % P else 0)  # 3 key blocks (128,128,64)
NPAIR = H // 2              # 8 head pairs == output chunks
TAUC = 384                  # tau cols: A[t0:128]=128 | B[t64:256]=192 | C=64
TAUP = 512                  # padded per-head pitch (full PSUM bank)

f32 = mybir.dt.float32
bf16 = mybir.dt.bfloat16
fp8 = mybir.dt.float8e4
DR = mybir.MatmulPerfMode.DoubleRow


def _build_program(loop_reps=None, stage=40):
    nc = bacc.Bacc("TRN2", target_bir_lowering=False, debug=False)

    def din(name, shape, dt):
        return nc.dram_tensor(name, list(shape), dt, kind="ExternalInput").ap()

    xh_d = din("xh_b", (TEXT, D), bf16)
    xo_d = din("xo_f", (P, 2, D), f32)
    wq_d = din("wq_b", (P, KCH, D), fp8)
    wk_d = din("wk_b", (P, KCH, D), fp8)
    wv_d = din("wv_b", (P, KCH, D), fp8)
    wo_d = din("wo_b", (P, KCH, D), fp8)
    identb_d = din("identb", (P, P), bf16)
    rotT_d = din("rotT", (P, P), bf16)
    cosq_d = din("cosq", (P, TLOC), bf16)
    sinq_d = din("sinq", (P, TLOC), f32)
    cosk_d = din("cosk", (P, TEXT), bf16)
    sink_d = din("sink", (P, TEXT), f32)
    masks_d = din("masks3", (P, 2, TAUC), bf16)
    emask_d = din("emaskT", (P, H, H), bf16)
    esel_d = din("esel", (H, MCH, P), bf16)
    nsc_d = din("norm_scale", (P, D), f32)
    y_d = nc.dram_tensor("y", [TLOC, D], f32, kind="ExternalOutput").ap()

    with tile.TileContext(nc) as tc:
        from contextlib import ExitStack
        with ExitStack() as ctx:
            if loop_reps is not None:
                loop = ctx.enter_context(tc.For_i(0, loop_reps, 1))
            sb = ctx.enter_context(tc.tile_pool(name="sb", bufs=1))
            sbw = ctx.enter_context(tc.tile_pool(name="sbw", bufs=4))
            sba = ctx.enter_context(tc.tile_pool(name="sba", bufs=3))
            sbt = ctx.enter_context(tc.tile_pool(name="sbt", bufs=3))
            sbz = ctx.enter_context(tc.tile_pool(name="sbz", bufs=2))
            # PSUM (8 banks of 2KB): pr 2 + ps 2x2 + pm 1 + pms 1 = 8.
            # HW rule: matmuls on disjoint PE row groups run concurrently
            # and FAULT if they write the same PSUM bank. Score matmuls for
            # the two heads of a pair sit on rows 0:64 / 64:128, so each
            # head's scores get their own full bank (TAUP=512 f32 pitch).
            # All kb2 matmuls (rows 0:64) write pm/pms banks while pair
            # score matmuls write ps banks - disjoint.
            pr = ctx.enter_context(tc.tile_pool(name="pr", bufs=2, space="PSUM"))
            ps = ctx.enter_context(tc.tile_pool(name="ps", bufs=2, space="PSUM"))
            pm = ctx.enter_context(tc.tile_pool(name="pm", bufs=1, space="PSUM"))
            pms = ctx.enter_context(tc.tile_pool(name="pms", bufs=1,
                                                 space="PSUM"))

            # ---- input DMAs (issue in consumption order) ----
            xeb_t = sb.tile([P, NKB, D], bf16, tag="xeb")
            nc.sync.dma_start(xeb_t[:, 0, :], xh_d[0:P])
            identb_t = sb.tile([P, P], bf16, tag="identb")
            nc.sync.dma_start(identb_t[:], identb_d)
            rot_t = sb.tile([P, P], bf16, tag="rot")
            nc.sync.dma_start(rot_t[:], rotT_d)
            nc.sync.dma_start(xeb_t[:, 1, :], xh_d[P:2 * P])
            nc.sync.dma_start(xeb_t[0:TEXT - 2 * P, 2, :], xh_d[2 * P:TEXT])

            # ---- PE warm-up on a memset tile (no DMA dependency): the
            # p-state clock needs ~3us of continuous execution to reach
            # full speed, so spin while the input DMAs land ----
            wuin = sb.tile([P, P], bf16, tag="wuin")
            nc.vector.memset(wuin[:], 0.5)
            # dummy sigmoid: forces the sigmoid_and_others ACT table load
            # (which also covers Copy) to happen NOW, overlapping the DMAs
            dum_t = sb.tile([1, 2], f32, tag="dum")
            nc.scalar.activation(dum_t[0:1, 0:1], wuin[0:1, 0:1],
                                 mybir.ActivationFunctionType.Sigmoid)
            wu_t = pr.tile([P, 512], f32, tag="pr", name="warmup")
            for _ in range(30):
                nc.tensor.matmul(wu_t[:, 0:P], wuin[:], wuin[:],
                                 start=True, stop=True)

            def load_w(wd, q1, q2):
                # host pre-transposed to [p, k, d]: fully contiguous DMA,
                # halves spread across two DGE queues
                wt = sbw.tile([P, KCH, D], fp8, tag="w")
                q1.dma_start(wt[:, 0:4, :], wd[:, 0:4, :])
                q2.dma_start(wt[:, 4:8, :], wd[:, 4:8, :])
                return wt

            cq_t = sb.tile([P, TLOC], bf16, tag="cq")
            sq_t = sb.tile([P, TLOC], f32, tag="sq")
            ck_t = sb.tile([P, TEXT], bf16, tag="ck")
            sk_t = sb.tile([P, TEXT], f32, tag="sk")
            nc.gpsimd.dma_start(cq_t[:], cosq_d)
            nc.gpsimd.dma_start(sq_t[:], sinq_d)
            nc.gpsimd.dma_start(ck_t[:], cosk_d)
            nc.gpsimd.dma_start(sk_t[:], sink_d)
            wv_t = load_w(wv_d, nc.gpsimd, nc.gpsimd)
            wq_t = load_w(wq_d, nc.gpsimd, nc.gpsimd)
            mask_t = sb.tile([P, 2, TAUC], bf16, tag="mask")
            nc.gpsimd.dma_start(mask_t[:], masks_d)
            emask_t = sb.tile([P, H, H], bf16, tag="emask")
            nc.gpsimd.dma_start(emask_t[:], emask_d)
            esel_t = sb.tile([H, MCH, P], bf16, tag="esel")
            nc.gpsimd.dma_start(esel_t[:], esel_d)
            wk_t = load_w(wk_d, nc.gpsimd, nc.gpsimd)
            xo_t = sb.tile([P, 2, D], f32, tag="xo")
            nc.scalar.dma_start(xo_t[:], xo_d)
            nsc_t = sb.tile([P, D], f32, tag="nsc")
            nc.gpsimd.dma_start(nsc_t[:], nsc_d)
            wo_t = load_w(wo_d, nc.gpsimd, nc.gpsimd)

            # ---- transpose bf16 x -> xT fp8 [dout, text] ----
            xT_t = sb.tile([P, KCH, TEXT], fp8, tag="xT")
            for b in range(NKB):
                rows = P if b < 2 else TEXT - 2 * P
                for g in range(2):
                    pt = pr.tile([P, 4, P], bf16, tag="pr", name=f"tp{b}{g}")
                    for j in range(4):
                        nc.tensor.transpose(
                            pt[:, j, 0:rows],
                            xeb_t[0:rows, b, (4 * g + j) * P:(4 * g + j + 1) * P],
                            identb_t[0:rows, 0:rows],
                        )
                    nc.scalar.activation(
                        xT_t[:, 4 * g:4 * g + 4, b * P:b * P + rows],
                        pt[:, :, 0:rows],
                        mybir.ActivationFunctionType.Copy,
                    )

            # ---- v projection (natural orientation, fp8 DoubleRow) ----
            v_t = sb.tile([P, NKB, D], bf16, tag="v")
            for b in range(NKB):
                rows = P if b < 2 else TEXT - 2 * P
                for half in range(2):
                    pv = pr.tile([P, 512], f32, tag="pr", name=f"pv{b}{half}")
                    for kp in range(KCH // 2):
                        nc.tensor.matmul(
                            pv[0:rows, :],
                            xT_t[:, 2 * kp:2 * kp + 2, b * P:b * P + rows],
                            wv_t[:, 2 * kp:2 * kp + 2,
                                 half * 512:(half + 1) * 512],
                            start=(kp == 0), stop=(kp == KCH // 2 - 1),
                            perf_mode=DR,
                        )
                    nc.scalar.activation(
                        v_t[0:rows, b, half * 512:(half + 1) * 512],
                        pv[0:rows, :], mybir.ActivationFunctionType.Copy)

            # ---- interleaved q/k projection chunks + attention pairs ----
            qT = sb.tile([P, MCH, TLOC], bf16, tag="qT")
            kT = sb.tile([P, MCH, TEXT], bf16, tag="kT")

            def proj_part(wt, m, ncols, col_off, which):
                """projection matmuls + PSUM->SBUF copy (PE + ACT)."""
                pq = pr.tile([P, ncols], f32, tag="pr", name=f"pq{which}{m}")
                for kp in range(KCH // 2):
                    nc.tensor.matmul(
                        pq[:, :],
                        wt[:, 2 * kp:2 * kp + 2, m * P:(m + 1) * P],
                        xT_t[:, 2 * kp:2 * kp + 2, col_off:col_off + ncols],
                        start=(kp == 0), stop=(kp == KCH // 2 - 1),
                        perf_mode=DR,
                    )
                a_t = sba.tile([P, TEXT], bf16, tag=f"a{which}",
                               name=f"a{which}{m}")
                nc.scalar.activation(a_t[:, 0:ncols], pq[:, :],
                                     mybir.ActivationFunctionType.Copy)
                return a_t

            def rope_part(m, ncols, a_t, cos_t, sin_t, outT):
                """rotation matmul + cos/sin combine (PE + DVE); consumes
                the a_t produced a full iteration earlier."""
                prt = pr.tile([P, ncols], f32, tag="pr", name=f"prot{m}")
                nc.tensor.matmul(prt[:, :], rot_t[:], a_t[:, 0:ncols],
                                 start=True, stop=True)
                t1 = sba.tile([P, TEXT], bf16, tag="t1")
                nc.vector.tensor_tensor(t1[:, 0:ncols], a_t[:, 0:ncols],
                                        cos_t[:], op=mybir.AluOpType.mult)
                t2 = sba.tile([P, TEXT], bf16, tag="t2")
                nc.vector.tensor_tensor(t2[:, 0:ncols], prt[:, :],
                                        sin_t[:], op=mybir.AluOpType.mult)
                nc.vector.tensor_tensor(outT[:, m, :], t1[:, 0:ncols],
                                        t2[:, 0:ncols], op=mybir.AluOpType.add)

            msgs = sb.tile([P, MCH, TLOC], bf16, tag="msgs")
            msgb = sb.tile([P, MCH, TLOC], fp8, tag="msgb")
            pms_t = pms.tile([16, TLOC], f32, tag="pms")
            mstate = {"i": 0}
            n_mass = 6 * NPAIR
            taus = {}

            # msg band matmuls: (kb, krows, tau col range, t range).
            # start=True only on the first (its psum-granule pending-zero
            # mark covers the whole 2KB row, so later partial-range
            # matmuls accumulate onto lazily-zeroed bytes).
            BAND = [(0, P, 0, 128, 0, 128),               # kb0 x t[0:128)
                    (1, P, 128, 320, 64, 256),            # kb1 x t[64:256)
                    (2, TEXT - 2 * P, 320, 384, 192, 256)]  # kb2 x t[192:)

            def emit_msg(c):
                """6 msg + 6 tau-mass matmuls for pair c; mass accumulates
                all 16 heads into one [16, TLOC] psum tile across pairs."""
                tau_t = taus.pop(c)
                pm_t = pm.tile([P, TLOC], f32, tag="pm", name=f"pm{c}")
                for hh in range(2):
                    h = 2 * c + hh
                    for j, (kb, krows, c0, c1, t0, t1_) in enumerate(BAND):
                        nc.tensor.matmul(
                            pm_t[hh * HD:(hh + 1) * HD, t0:t1_],
                            v_t[0:krows, kb, h * HD:(h + 1) * HD],
                            tau_t[0:krows, hh, c0:c1],
                            start=(j == 0), stop=(j == 2),
                            tile_position=(0, hh * HD),
                            skip_group_check=True,
                        )
                if stage >= 40:
                    for hh in range(2):
                        h = 2 * c + hh
                        for j, (kb, krows, c0, c1, t0, t1_) in enumerate(BAND):
                            nc.tensor.matmul(
                                pms_t[:, t0:t1_],
                                emask_t[0:krows, h, :],
                                tau_t[0:krows, hh, c0:c1],
                                start=(mstate["i"] == 0),
                                stop=(mstate["i"] == n_mass - 1),
                                skip_group_check=True,
                            )
                            mstate["i"] += 1
                with nc.allow_low_precision(reason="bf16 msg pre-normalize"):
                    nc.scalar.activation(msgs[:, c, :], pm_t[:],
                                         mybir.ActivationFunctionType.Copy)

            def attn_block(c):
                """scores + sigmoid + mask for pair c."""
                ps_t = ps.tile([P, 2, TAUP], f32, tag="ps", name=f"ps{c}")
                for hh in range(2):
                    po = hh * HD
                    # s1: kb0 x t[0:128]; s2: kb1 x t[64:256]; s3: kb2 x
                    # t[192:256] (keys block 2 has 64 rows)
                    nc.tensor.matmul(
                        ps_t[:, hh, 0:128],
                        kT[po:po + HD, c, 0:P],
                        qT[po:po + HD, c, 0:128],
                        start=True, stop=True)
                    nc.tensor.matmul(
                        ps_t[:, hh, 128:320],
                        kT[po:po + HD, c, P:2 * P],
                        qT[po:po + HD, c, 64:256],
                        start=True, stop=True)
                    nc.tensor.matmul(
                        ps_t[0:TEXT - 2 * P, hh, 320:384],
                        kT[po:po + HD, c, 2 * P:TEXT],
                        qT[po:po + HD, c, 192:256],
                        start=True, stop=True)
                tau_t = sbt.tile([P, 2, TAUC], bf16, tag="tau", name=f"tau{c}")
                taus[c] = tau_t
                if stage >= 12:
                    # single op over the full tile; the unwritten psum
                    # corner [64:128, :, 320:384] is finite garbage and is
                    # zeroed by the mask below (mask=0 there)
                    nc.scalar.activation(
                        tau_t[:, :, :],
                        ps_t[:, :, 0:TAUC],
                        mybir.ActivationFunctionType.Sigmoid)
                if stage >= 20:
                    nc.vector.tensor_tensor(tau_t[:, :, :],
                                            tau_t[:, :, :],
                                            mask_t[:, :, :],
                                            op=mybir.AluOpType.mult)
                if c == NPAIR - 1 and stage >= 12:
                    # dummy sqrt: pull the sqrt_and_others table load off
                    # the output-phase critical path. Reads the LAST tau so
                    # the scheduler cannot hoist it before the sigmoids
                    # (which would evict the sigmoid table mid-stream).
                    nc.scalar.activation(dum_t[0:1, 1:2], tau_t[0:1, 0, 0:1],
                                         mybir.ActivationFunctionType.Sqrt)

            # Lookahead pipeline: iteration c issues rope for pair c-1
            # (from a_t written last iteration), projection matmuls+copies
            # for pair c, msg+mass for pair c-2, and scores/sigmoid/mask
            # for pair c-1 (rope'd at this iteration's start, with all of
            # proj+emit in between as slack). No block waits on a
            # same-iteration PE<->ACT round trip, so the PE stays busy and
            # its p-state clock stays up.
            aqs, aks = {}, {}
            for c in range(NPAIR):
                if c >= 1:
                    rope_part(c - 1, TLOC, aqs.pop(c - 1), cq_t, sq_t, qT)
                    rope_part(c - 1, TEXT, aks.pop(c - 1), ck_t, sk_t, kT)
                aqs[c] = proj_part(wq_t, c, TLOC, WN, "q")
                aks[c] = proj_part(wk_t, c, TEXT, 0, "k")
                if stage < 11:
                    continue
                if stage >= 30 and c >= 2:
                    emit_msg(c - 2)
                if c >= 1:
                    attn_block(c - 1)
            rope_part(NPAIR - 1, TLOC, aqs.pop(NPAIR - 1), cq_t, sq_t, qT)
            rope_part(NPAIR - 1, TEXT, aks.pop(NPAIR - 1), ck_t, sk_t, kT)
            if stage >= 11:
                if stage >= 30:
                    emit_msg(NPAIR - 2)
                attn_block(NPAIR - 1)
            if stage >= 30:
                emit_msg(NPAIR - 1)
                # ---- tau-mass normalization (single end-phase pass) ----
                rinv_t = sb.tile([16, TLOC], bf16, tag="rinv")
                if stage >= 40:
                    mass_sb = sb.tile([16, TLOC], f32, tag="mass")
                    nc.vector.tensor_scalar_max(mass_sb[:], pms_t[:], 1e-6)
                    rinvf = sb.tile([16, TLOC], f32, tag="rinvf")
                    nc.vector.reciprocal_approx_fast(rinvf[:], mass_sb[:])
                    with nc.allow_low_precision(reason="bf16 1/mass is fine"):
                        nc.vector.tensor_copy(rinv_t[:], rinvf[:])
                else:
                    nc.vector.memset(rinv_t[:], 1.0)
                for c in range(NPAIR):
                    pe = pr.tile([P, TLOC], f32, tag="pr", name=f"pe{c}")
                    nc.tensor.matmul(pe[:, :], esel_t[:, c, :], rinv_t[:],
                                     start=True, stop=True)
                    with nc.allow_low_precision(reason="fp8 msg for wo"):
                        nc.vector.tensor_tensor(msgb[:, c, :], msgs[:, c, :],
                                                pe[:, :],
                                                op=mybir.AluOpType.mult)
            else:
                nc.vector.memset(msgb[:], 0.01)

            # ---- output projection + residual + rms norm ----
            for t2 in range(2):
                z_t = sbz.tile([P, D], f32, tag="z")
                for half in range(2):
                    pz = pr.tile([P, 512], f32, tag="pr", name=f"pz{t2}{half}")
                    for kp in range(KCH // 2):
                        nc.tensor.matmul(
                            pz[:, :],
                            msgb[:, 2 * kp:2 * kp + 2, t2 * P:(t2 + 1) * P],
                            wo_t[:, 2 * kp:2 * kp + 2,
                                 half * 512:(half + 1) * 512],
                            start=(kp == 0), stop=(kp == KCH // 2 - 1),
                            perf_mode=DR,
                        )
                    nc.vector.tensor_tensor(
                        z_t[:, half * 512:(half + 1) * 512],
                        pz[:, :], xo_t[:, t2, half * 512:(half + 1) * 512],
                        op=mybir.AluOpType.add)
                z2 = sbz.tile([P, D], f32, tag="zs", name="z2")
                ssq = sbz.tile([P, 1], f32, tag="ssq")
                nc.vector.scalar_tensor_tensor(
                    z2[:], z_t[:], 1.0, z_t[:],
                    op0=mybir.AluOpType.bypass, op1=mybir.AluOpType.mult,
                    accum_out=ssq[:])
                nc.vector.tensor_scalar(ssq[:], ssq[:], D * EPS, None,
                                        op0=mybir.AluOpType.add)
                sroot = sbz.tile([P, 1], f32, tag="sroot")
                nc.scalar.activation(sroot[:], ssq[:],
                                     mybir.ActivationFunctionType.Sqrt)
                rinv2 = sbz.tile([P, 1], f32, tag="rinv2")
                nc.vector.reciprocal(rinv2[:], sroot[:])
                out_t = sbz.tile([P, D], f32, tag="zs", name="out_t")
                # out = (z * rinv2) * nsc in one fused DVE op
                nc.vector.scalar_tensor_tensor(
                    out_t[:], z_t[:], rinv2[:], nsc_t[:],
                    op0=mybir.AluOpType.mult, op1=mybir.AluOpType.mult)
                nc.sync.dma_start(y_d[t2 * P:(t2 + 1) * P, :], out_t[:])

    nc.compile()
    return nc


def _host_tables():
    """Core-independent constant inputs."""
    half = HD // 2
    bft = mybir.dt.np(bf16)
    identb = np.eye(P, dtype=np.float32).astype(bft)
    # Rot = blockdiag(J, J) with J = [[0, -I32], [I32, 0]] on 64-row groups
    rot = np.zeros((P, P), dtype=np.float32)
    for g in range(2):
        o = g * 64
        for r in range(half):
            rot[o + r, o + half + r] = -1.0
            rot[o + half + r, o + r] = 1.0
    rotT = rot.T.copy().astype(bft)
    emask = np.zeros((P, H, H), dtype=np.float32)
    for h in range(H):
        emask[:, h, h] = 1.0
    esel = np.zeros((H, MCH, P), dtype=np.float32)
    for c in range(MCH):
        esel[2 * c, c, 0:HD] = 1.0
        esel[2 * c + 1, c, HD:P] = 1.0
    return identb, rotT, emask.astype(bft), esel.astype(bft)


def _trig(positions: np.ndarray, scale: float):
    """cos/sin tables tiled to [128, len(positions)], both bf16."""
    half = HD // 2
    bft = mybir.dt.np(bf16)
    theta = 1.0 / (ROPE_BASE ** (np.arange(half, dtype=np.float64) / half))
    freqs = positions[:, None].astype(np.float64) * theta[None, :]  # [n, 32]
    c = (np.cos(freqs).T * scale).astype(np.float32)  # [32, n]
    s = (np.sin(freqs).T * scale).astype(np.float32)
    return np.tile(c, (4, 1)).astype(bft), np.tile(s, (4, 1))


def _masks(core: int) -> np.ndarray:
    """[P, 2, TAUC] bf16 mask in the banded tau layout.

    tau col ranges: A=[0:128) keys kb0 x t[0:128); B=[128:320) keys kb1 x
    t[64:256); C=[320:384) keys kb2 (ext rows 256:320) x t[192:256).
    mask=1 iff key ext row is in query t's window (and causally valid for
    core 0, whose first WN halo rows are zero-padding).
    """
    m = np.zeros((P, TAUC), dtype=np.float32)
    segs = [(0, 0, 0, 128), (1, 128, 64, 256), (2, 320, 192, 256)]
    for kb, c0, t0, t1 in segs:
        j = np.arange(P)[:, None]          # key row within block
        t = np.arange(t0, t1)[None, :]     # query t
        w = 128 * kb + j - t
        valid = (w >= 0) & (w < WN)
        if core == 0:
            valid &= (128 * kb + j) >= WN
        m[:, c0:c0 + (t1 - t0)] = valid.astype(np.float32)
    m2 = np.broadcast_to(m[:, None, :], (P, 2, TAUC))
    return np.ascontiguousarray(m2).astype(mybir.dt.np(bf16))


_CACHE = {}


def _make_runner(nc):
    """Persistent sharded-jit executor over the 8 cores (mirrors
    bass2jax.run_bass_via_pjrt's multi-core path, but reusable so repeat
    calls skip retracing/recompilation)."""
    import jax
    from jax.experimental.shard_map import shard_map
    from jax.sharding import Mesh, PartitionSpec
    from concourse import bass2jax

    bass2jax.install_neuronx_cc_hook()
    partition_name = (nc.partition_id_tensor.name
                      if nc.partition_id_tensor else None)
    in_names, out_names, out_avals = [], [], []
    for alloc in nc.m.functions[0].allocations:
        if not isinstance(alloc, mybir.MemoryLocationSet):
            continue
        if alloc.kind not in ("ExternalInput", "ExternalOutput"):
            continue
        name = alloc.memorylocations[0].name
        if alloc.kind == "ExternalInput":
            if name != partition_name:
                in_names.append(name)
        else:
            out_names.append(name)
            out_avals.append(jax.core.ShapedArray(
                tuple(alloc.tensor_shape), mybir.dt.np(alloc.dtype)))
    n_params, n_outs = len(in_names), len(out_names)
    bind_names = in_names + out_names + (
        [partition_name] if partition_name else [])

    def _body(*args):
        operands = list(args)
        if partition_name is not None:
            operands.append(bass2jax.partition_id_tensor())
        outs = bass2jax._bass_exec_p.bind(
            *operands,
            out_avals=tuple(out_avals),
            in_names=tuple(bind_names),
            out_names=tuple(out_names),
            lowering_input_output_aliases=(),
            sim_require_finite=True,
            sim_require_nnan=True,
            nc=nc,
        )
        return tuple(outs)

    devices = jax.devices()[:NCORES]
    mesh = Mesh(np.asarray(devices), ("core",))
    sharded = jax.jit(
        shard_map(_body, mesh=mesh,
                  in_specs=(PartitionSpec("core"),) * (n_params + n_outs),
                  out_specs=(PartitionSpec("core"),) * n_outs,
                  check_rep=False),
        donate_argnums=tuple(range(n_params, n_params + n_outs)),
        keep_unused=True)

    def run(in_maps):
        concat_in = [np.concatenate([m[name] for m in in_maps], axis=0)
                     for name in in_names]
        zeros = [np.zeros((NCORES * a.shape[0], *a.shape[1:]), a.dtype)
                 for a in out_avals]
        out_arrs = sharded(*concat_in, *zeros)
        return [
            {name: np.asarray(out_arrs[i]).reshape(
                NCORES, *out_avals[i].shape)[c]
             for i, name in enumerate(out_names)}
            for c in range(NCORES)
        ]

    run.sharded = sharded
    run.in_names = in_names
    run.out_names = out_names
    run.out_avals = out_avals
    return run


def _in_maps(x, wq, wk, wv, wo, norm_scale):
    f8t = mybir.dt.np(fp8)
    bft = mybir.dt.np(bf16)
    identb, rotT, emask, esel = _host_tables()

    def wprep(w):
        w8 = np.asarray(w, dtype=np.float32).astype(f8t)
        return np.ascontiguousarray(w8.reshape(KCH, P, D).transpose(1, 0, 2))

    wq_b, wk_b, wv_b, wo_b = wprep(wq), wprep(wk), wprep(wv), wprep(wo)
    nsc = np.ascontiguousarray(
        np.broadcast_to(np.asarray(norm_scale, dtype=np.float32)
                        * np.float32(np.sqrt(D)), (P, D)))

    xf = np.asarray(x, dtype=np.float32).reshape(T, D)
    in_maps = []
    for c in range(NCORES):
        t0 = c * TLOC
        x_halo = np.zeros((TEXT, D), dtype=np.float32)
        lo = max(0, t0 - WN)
        x_halo[WN - (t0 - lo):] = xf[lo:t0 + TLOC]
        cosq, sinq = _trig(np.arange(t0, t0 + TLOC), 1.0 / 8.0)
        cosk, sink = _trig(np.arange(t0 - WN, t0 + TLOC), 1.0)
        in_maps.append({
            "xh_b": x_halo.astype(bft),
            "xo_f": np.ascontiguousarray(
                x_halo[WN:].reshape(2, P, D).transpose(1, 0, 2)),
            "wq_b": wq_b, "wk_b": wk_b, "wv_b": wv_b, "wo_b": wo_b,
            "identb": identb, "rotT": rotT,
            "cosq": cosq, "sinq": sinq, "cosk": cosk, "sink": sink,
            "masks3": _masks(c), "emaskT": emask, "esel": esel,
            "norm_scale": nsc,
        })
    return in_maps


def kernel(x, wq, wk, wv, wo, norm_scale):
    if "nc" not in _CACHE:
        _CACHE["nc"] = _build_program()
        _CACHE["runner"] = _make_runner(_CACHE["nc"])
    nc = _CACHE["nc"]
    in_maps = _in_maps(x, wq, wk, wv, wo, norm_scale)
    _CACHE["last_in_maps"] = in_maps
    if "first_done" not in _CACHE:
        res = bass_utils.run_bass_kernel_spmd(
            nc, in_maps, core_ids=list(range(NCORES)))
        results = res.results
        _CACHE["first_done"] = True
    else:
        results = _CACHE["runner"](in_maps)
    out = np.empty((1, T, D), dtype=np.float32)
    for c in range(NCORES):
        out[0, c * TLOC:(c + 1) * TLOC] = results[c]["y"]
    return out
